# revision 1
# baseline (speedup 1.0000x reference)
"""Causal self-attention (B=2, T=2048, D=1024, H=16, HD=64) on 8 trn2 NeuronCores.

Sharding: core = b*4 + g  (b = batch 0..1, g = head-group 0..3, 4 heads each).
Megatron-style: column-split w_qkv per head group, row-split w_o; the w_o
all-reduce over each batch's 4 cores is done host-side (partial sums).

Per-core device program (Tile framework, fp32r matmuls):
  1. qkT = w_qk_g^T-free matmul: qk[dq, t] (transposed layout, dq = 4h*64 q
     rows then 4h*64 k rows), bias added per-partition at PSUM eviction.
  2. v[t, dv_ext] natural layout; dv_ext = per head [64 v cols | 1.0 ones col]
     (ones col from zero weight col + bias 1.0) -> lhsT for the y matmul also
     yields the softmax denominator for free.
  3. Per (head, tq-chunk 512): scoresT[tk,tq] blocks (K=64), exp on ACT with
     scale 1/32 (softmax max-subtraction is provably unnecessary: scores/32
     has std ~0.25), causal mask via gpsimd affine_select on diagonal pairs,
     y^T accumulation in PSUM with denom row, reciprocal + partition
     broadcast + multiply to normalize.
  4. out_part[t, :] = y_local^T.T @ w_o_g rows; host sums the 4 partials per
     batch and adds b_o.
"""

import os
import numpy as np

B, T, D = 2, 2048, 1024
H, HD = 16, 64
LH = 4            # local heads per core
KO = 8            # contraction tiles of 128 over D
DQK = 512         # q+k columns per core (4 heads * 64 * 2)
DVE_ = 260        # v columns per core incl. ones cols (4 * 65)
NTQ, TQC = 4, 512
NTKB, TKB = 16, 128

_PROG = {}
LAST_RESULT = None


def _build_program(debug_dumps=False):
    import concourse.bass as bass
    from concourse import bacc
    import concourse.tile as tile
    import concourse.mybir as mybir

    f32 = mybir.dt.float32
    f32r = mybir.dt.float32r
    AF = mybir.ActivationFunctionType
    ALU = mybir.AluOpType
    ts = bass.ts

    nc = bacc.Bacc(None, target_bir_lowering=False, debug=True)
    xT_d = nc.dram_tensor("xT", [128, KO, T], f32, kind="ExternalInput")
    wqk_d = nc.dram_tensor("w_qk", [128, KO, DQK], f32, kind="ExternalInput")
    bqk_d = nc.dram_tensor("b_qk", [128, 4], f32, kind="ExternalInput")
    wv_d = nc.dram_tensor("w_v", [128, KO, DVE_], f32, kind="ExternalInput")
    bv_d = nc.dram_tensor("b_v", [128, DVE_], f32, kind="ExternalInput")
    wo_d = nc.dram_tensor("w_o", [128, 2, D], f32, kind="ExternalInput")
    zeros_d = nc.dram_tensor("zeros64", [64, T], f32, kind="ExternalInput")
    ident_d = nc.dram_tensor("ident", [128, 128], f32, kind="ExternalInput")
    masks_d = nc.dram_tensor("masks", [128, 896], f32, kind="ExternalInput")
    out_d = nc.dram_tensor("out_part", [T, D], f32, kind="ExternalOutput")
    dbg = {}
    if debug_dumps:
        dbg["qk"] = nc.dram_tensor("dbg_qk", [128, 2 * LH, T], f32, kind="ExternalOutput")
        dbg["v"] = nc.dram_tensor("dbg_v", [128, NTKB, LH, 65], f32, kind="ExternalOutput")
        dbg["yT"] = nc.dram_tensor("dbg_yT", [128, 2, T], f32, kind="ExternalOutput")
        dbg["et"] = nc.dram_tensor("dbg_et", [128, 2, 512], f32, kind="ExternalOutput")

    with tile.TileContext(nc) as tc:
        with (
            tc.tile_pool(name="big", bufs=1) as big,
            tc.tile_pool(name="xtp", bufs=2) as xtp,
            tc.tile_pool(name="stg", bufs=3) as stg,
            tc.tile_pool(name="expp", bufs=3) as expp,
            tc.tile_pool(name="ev", bufs=2) as ev,
            tc.tile_pool(name="outp", bufs=3) as outp,
            tc.tile_pool(name="ps_misc", bufs=2, space="PSUM") as ps_misc,
            tc.tile_pool(name="ps_s", bufs=2, space="PSUM") as ps_sp,
            tc.tile_pool(name="ps_y", bufs=2, space="PSUM") as ps_yp,
        ):
            wqk = big.tile([128, KO, DQK], f32r, name="wqk_sb")
            wv = big.tile([128, KO, DVE_], f32r, name="wv_sb")
            wo = big.tile([128, 2, D], f32r, name="wo_sb")
            bqk = big.tile([128, 4], f32, name="bqk_sb")
            bv = big.tile([128, LH, 65], f32, name="bv_sb")
            # qk_pad: subtiles 0-3 = q heads 0-3, 4-7 = k heads 0-3;
            # rows 64:128 zeroed so attention matmuls contract over K=128
            qk = big.tile([128, 2 * LH, T], f32r, name="qk_sb")
            vsb = big.tile([128, NTKB, LH, 65], f32r, name="v_sb")
            yT = big.tile([128, 2, T], f32r, name="yT_sb")

            ident = big.tile([128, 128], f32r, name="ident_sb")
            msk = big.tile([128, 896], f32r, name="msk_sb")
            # startup DMA order = first-need order: wqk + x chunk 0 gate the
            # first matmul; zeros/masks only gate attention
            nc.sync.dma_start(wqk[:], wqk_d[:].bitcast(f32r))

            for c in range(NTQ):
                # chunk load: x^T columns [512c, 512c+512)
                xTc = xtp.tile([128, KO, 512], f32r, name=f"xTc_{c}", tag="xTc")
                for ko in range(KO):
                    nc.sync.dma_start(
                        xTc[:, ko, :], xT_d[:, ko, ts(c, 512)].bitcast(f32r)
                    )
                if c == 0:
                    nc.sync.dma_start(bqk[:], bqk_d[:])
                    nc.sync.dma_start(wv[:], wv_d[:].bitcast(f32r))
                    nc.sync.dma_start(bv[:], bv_d[:].rearrange("p (h e) -> p h e", h=LH))
                    nc.sync.dma_start(ident[:], ident_d[:].bitcast(f32r))
                    nc.sync.dma_start(msk[:], masks_d[:].bitcast(f32r))
                    for su in range(2 * LH):
                        nc.sync.dma_start(qk[64:128, su, :], zeros_d[:].bitcast(f32r))
                    nc.sync.dma_start(wo[:], wo_d[:].bitcast(f32r))

                # q,k projection for this chunk -> padded per-head layout
                for s in range(4):
                    pst = ps_misc.tile([128, 512], f32, name=f"ps1_{s}_{c}", tag="misc")
                    for ko in range(KO):
                        nc.tensor.matmul(
                            pst[:],
                            wqk[:, ko, ts(s, 128)],
                            xTc[:, ko, :],
                            start=(ko == 0),
                            stop=(ko == KO - 1),
                        )
                    sg = stg.tile([128, 512], f32r, name=f"sg_{s}_{c}", tag="sg")
                    nc.vector.tensor_scalar_add(sg[:], pst[:], bqk[:, s : s + 1])
                    # s 0,1 = q head pairs (0,1),(2,3); s 2,3 = k head pairs
                    h0 = (0 if s < 2 else LH) + 2 * (s % 2)
                    nc.gpsimd.dma_start(qk[0:64, h0, ts(c, 512)], sg[0:64, :])
                    nc.gpsimd.dma_start(qk[0:64, h0 + 1, ts(c, 512)], sg[64:128, :])

                # v projection for tk blocks of this chunk (with ones column)
                for tbl in range(4):
                    tb = 4 * c + tbl
                    pst = ps_misc.tile([128, DVE_], f32, name=f"ps2_{tb}", tag="misc")
                    for ko in range(KO):
                        nc.tensor.matmul(
                            pst[:],
                            xTc[:, ko, ts(tbl, 128)],
                            wv[:, ko, :],
                            start=(ko == 0),
                            stop=(ko == KO - 1),
                        )
                    nc.vector.tensor_add(
                        vsb[:, tb, :, :],
                        pst[:].rearrange("p (h e) -> p h e", h=LH),
                        bv[:],
                    )

                # attention for tq chunk c: head pairs in lockstep so PE
                # has the other head's scores to run while ACT does exp
                for hp in range(2):
                    nb = 4 * (c + 1)
                    lhs = (2 * hp, 2 * hp + 1)
                    psys = {}
                    for lh in lhs:
                        psys[lh] = ps_yp.tile(
                            [128, 512], f32, name=f"psy_{lh}_{c}", tag="psy"
                        )
                    for j2 in range(nb // 2):
                        ets = {}
                        for lh in lhs:
                            pss = ps_sp.tile(
                                [128, 2, 512], f32, name=f"pss_{lh}_{c}_{j2}", tag="pss"
                            )
                            for j in range(2):
                                tb = 2 * j2 + j
                                off = 512 * c - 128 * tb
                                diag = -384 <= off <= 0
                                nc.tensor.matmul(
                                    pss[:, j, :],
                                    qk[:, LH + lh, ts(tb, 128)],
                                    qk[:, lh, ts(c, 512)],
                                    start=True,
                                    stop=not diag,
                                )
                                if diag:
                                    nc.tensor.matmul(
                                        pss[:, j, :],
                                        ident[:],
                                        msk[:, 384 + off : 896 + off],
                                        start=False,
                                        stop=True,
                                    )
                            et = expp.tile(
                                [128, 2, 512], f32r, name=f"et_{lh}_{c}_{j2}", tag="et"
                            )
                            nc.scalar.activation(et[:], pss[:], AF.Exp, scale=1.0 / 32.0)
                            ets[lh] = et
                        for lh in lhs:
                            for j in range(2):
                                tb = 2 * j2 + j
                                nc.tensor.matmul(
                                    psys[lh][0:65, :],
                                    vsb[:, tb, lh, :],
                                    ets[lh][:, j, :],
                                    start=(tb == 0),
                                    stop=(tb == nb - 1),
                                )
                    for lh in lhs:
                        p0 = (lh % 2) * 64
                        psy = psys[lh]
                        rc = ev.tile([128, 512], f32, name=f"rc_{lh}_{c}", tag="rc")
                        nc.vector.reciprocal(rc[64:65, :], psy[64:65, :])
                        rc0 = ev.tile([1, 512], f32, name=f"rc0_{lh}_{c}", tag="rc0")
                        nc.sync.dma_start(rc0[:], rc[64:65, :])
                        rb = ev.tile([128, 512], f32, name=f"rb_{lh}_{c}", tag="rb")
                        nc.gpsimd.partition_broadcast(rb[0:64, :], rc0[:])
                        kt_y = lh // 2
                        if p0 == 0:
                            nc.vector.tensor_mul(
                                yT[0:64, kt_y, ts(c, 512)], psy[0:64, :], rb[0:64, :]
                            )
                        else:
                            tmp = ev.tile(
                                [64, 512], f32r, name=f"tmp_{lh}_{c}", tag="tmpy"
                            )
                            nc.vector.tensor_mul(tmp[:], psy[0:64, :], rb[0:64, :])
                            nc.gpsimd.dma_start(yT[64:128, kt_y, ts(c, 512)], tmp[:])

                # output projection for the 4 t-blocks of this chunk
                for mb in range(4):
                    m = 4 * c + mb
                    for n in range(2):
                        pst = ps_misc.tile(
                            [128, 512], f32, name=f"ps4_{m}_{n}", tag="misc"
                        )
                        for kt in range(2):
                            nc.tensor.matmul(
                                pst[:],
                                yT[:, kt, ts(m, 128)],
                                wo[:, kt, ts(n, 512)],
                                start=(kt == 0),
                                stop=(kt == 1),
                            )
                        ot = outp.tile([128, 512], f32, name=f"ot_{m}_{n}", tag="ot")
                        if n == 0:
                            nc.scalar.copy(ot[:], pst[:])
                        else:
                            nc.vector.tensor_copy(ot[:], pst[:])
                        nc.sync.dma_start(out_d[ts(m, 128), ts(n, 512)], ot[:])

            if debug_dumps:
                nc.sync.dma_start(dbg["qk"][:], qk[:].bitcast(f32))
                nc.sync.dma_start(dbg["v"][:], vsb[:].bitcast(f32))
                nc.sync.dma_start(dbg["yT"][:], yT[:].bitcast(f32))

    nc.finalize()
    return nc


def _mask_tiles():
    # sliding causal mask: M[p, g] = -1e30 iff g < p + 384; a diagonal block
    # with offset off = tq0-tk0 in {0,-128,-256,-384} uses slice
    # M[:, 384+off : 896+off] so masked iff f + off < p
    p = np.arange(128)[:, None]
    g = np.arange(896)[None, :]
    return np.ascontiguousarray(np.where(g < p + 384, -1e30, 0.0).astype(np.float32))


def kernel(x, w_qkv, b_qkv, w_o, b_o):
    global LAST_RESULT
    from concourse.bass_utils import run_bass_kernel_spmd

    x = np.asarray(x, dtype=np.float32)
    w_qkv = np.asarray(w_qkv, dtype=np.float32)
    b_qkv = np.asarray(b_qkv, dtype=np.float32)
    w_o = np.asarray(w_o, dtype=np.float32)
    b_o = np.asarray(b_o, dtype=np.float32)

    if "nc" not in _PROG:
        _PROG["nc"] = _build_program()
    nc = _PROG["nc"]

    # host-side shard prep
    xT = []
    for b in range(B):
        t = np.ascontiguousarray(x[b].T)  # [D, T]
        xT.append(np.ascontiguousarray(t.reshape(KO, 128, T).swapaxes(0, 1)))

    in_maps = []
    for core in range(8):
        b, g = divmod(core, 4)
        qcols = slice(g * 256, (g + 1) * 256)
        kcols = slice(D + g * 256, D + (g + 1) * 256)
        w_qk = np.concatenate([w_qkv[:, qcols], w_qkv[:, kcols]], axis=1)  # [D, 512]
        w_qk = np.ascontiguousarray(w_qk.reshape(KO, 128, DQK).swapaxes(0, 1))
        b_qk = np.concatenate([b_qkv[qcols], b_qkv[kcols]])  # [512]
        b_qk = np.ascontiguousarray(b_qk.reshape(4, 128).T)  # [128, 4]

        w_v = np.zeros((D, DVE_), dtype=np.float32)
        b_v = np.zeros((DVE_,), dtype=np.float32)
        for h in range(LH):
            vcols = slice(2 * D + g * 256 + h * 64, 2 * D + g * 256 + (h + 1) * 64)
            w_v[:, h * 65 : h * 65 + 64] = w_qkv[:, vcols]
            b_v[h * 65 : h * 65 + 64] = b_qkv[vcols]
            b_v[h * 65 + 64] = 1.0  # ones column (weight col stays 0)
        w_v = np.ascontiguousarray(w_v.reshape(KO, 128, DVE_).swapaxes(0, 1))
        b_v_bc = np.ascontiguousarray(np.tile(b_v[None, :], (128, 1)))

        w_o_g = w_o[g * 256 : (g + 1) * 256, :]  # [256, D]
        w_o_g = np.ascontiguousarray(w_o_g.reshape(2, 128, D).swapaxes(0, 1))

        in_maps.append(
            {
                "xT": xT[b],
                "w_qk": w_qk,
                "b_qk": b_qk,
                "w_v": w_v,
                "b_v": b_v_bc,
                "w_o": w_o_g,
                "zeros64": np.zeros((64, T), dtype=np.float32),
                "ident": np.eye(128, dtype=np.float32),
                "masks": _mask_tiles(),
            }
        )

    trace = bool(os.environ.get("KERNEL_TRACE"))
    res = run_bass_kernel_spmd(nc, in_maps, core_ids=list(range(8)), trace=trace)
    LAST_RESULT = res

    out = np.empty((B, T, D), dtype=np.float32)
    for b in range(B):
        acc = res.results[b * 4]["out_part"].astype(np.float32).copy()
        for g in range(1, 4):
            acc += res.results[b * 4 + g]["out_part"]
        out[b] = acc + b_o[None, :]
    return out



# revision 11
# speedup vs baseline: 1.3562x; 1.3562x over previous
"""Causal self-attention (B=2, T=2048, D=1024, H=16, HD=64) on 8 trn2 NeuronCores.

Sharding: core = b*4 + g  (b = batch 0..1, g = head-group 0..3, 4 heads each).
Megatron-style: column-split w_qkv per head group, row-split w_o; the w_o
all-reduce over each batch's 4 cores is done host-side (partial sums).

v2 design (vs. fp32r baseline at ~315us):
  - all matmul operands bf16 (halves DMA, SBUF; same 1 cycle/row on PE)
  - heads packed 2-per-128-partitions; scores matmuls contract K=64 at
    partition bases {0,64} via tile_position -> no zero padding, and q/k
    projection evictions write SBUF directly (no shift DMAs)
  - causal mask applied as 0/1 bf16 multiply on DVE after exp (off PE)
  - softmax denominator via ones-column in v (row 64 of yacc PSUM);
    normalization = tensor_tensor divide on DVE fed by a gpsimd
    partition_broadcast (no 3.3us DVE reciprocals)
  - software-pipelined PE stream: scores(j)/exp(j) lookahead-1 ahead of
    yacc(j-1); next-chunk projections and prev-chunk out-projections are
    injected between attention matmuls so PE never drains (keeps p-state
    at 2.4 GHz)
  - DMAs split across sync (x loads) and gpsimd (weights, denom moves,
    yT odd-head shifts, out stores) queues
"""

import os
from collections import deque

import numpy as np

B, T, D = 2, 2048, 1024
H, HD = 16, 64
LH = 4            # local heads per core
KO = 8            # contraction tiles of 128 over D
NTQ, TQC = 4, 512  # tq chunks
DVE_ = 4 * 65      # v cols incl. ones col per head

_PROG = {}
LAST_RESULT = None


def _build_program(debug_dumps=False):
    import concourse.bass as bass
    from concourse import bacc
    import concourse.tile as tile
    import concourse.mybir as mybir

    f32 = mybir.dt.float32
    bf16 = mybir.dt.bfloat16
    AF = mybir.ActivationFunctionType
    ALU = mybir.AluOpType
    ts = bass.ts

    nc = bacc.Bacc(None, target_bir_lowering=False, debug=True)
    xT_d = nc.dram_tensor("xT", [128, KO, T], bf16, kind="ExternalInput")
    wqk_d = nc.dram_tensor("w_qk", [128, KO, 4, 128], bf16, kind="ExternalInput")
    bqk_d = nc.dram_tensor("b_qk", [128, 4], f32, kind="ExternalInput")
    wv_d = nc.dram_tensor("w_v", [128, KO, DVE_], bf16, kind="ExternalInput")
    bv_d = nc.dram_tensor("b_v", [128, LH, 65], f32, kind="ExternalInput")
    wo_d = nc.dram_tensor("w_o", [128, 2, D], bf16, kind="ExternalInput")
    msk_d = nc.dram_tensor("masks", [128, 4, TQC], bf16, kind="ExternalInput")
    out_d = nc.dram_tensor("out_part", [T, D], f32, kind="ExternalOutput")
    dbg = {}
    if debug_dumps:
        dbg["qk"] = nc.dram_tensor("dbg_qk", [128, 4, T], bf16, kind="ExternalOutput")
        dbg["v"] = nc.dram_tensor("dbg_v", [128, 4 * NTQ, LH, 65], bf16, kind="ExternalOutput")
        dbg["yT"] = nc.dram_tensor("dbg_yT", [128, 2, T], bf16, kind="ExternalOutput")
        dbg["et"] = nc.dram_tensor("dbg_et", [128, 2, TQC], bf16, kind="ExternalOutput")
        dbg["etm"] = nc.dram_tensor("dbg_etm", [128, 2, TQC], bf16, kind="ExternalOutput")
        dbg["d0"] = nc.dram_tensor("dbg_d0", [16, TQC], f32, kind="ExternalOutput")
        dbg["d0b"] = nc.dram_tensor("dbg_d0b", [16, TQC], f32, kind="ExternalOutput")
        dbg["d0c"] = nc.dram_tensor("dbg_d0c", [16, TQC], f32, kind="ExternalOutput")

    with tile.TileContext(nc) as tc:
        with (
            tc.tile_pool(name="big", bufs=1) as big,
            tc.tile_pool(name="xtp", bufs=2) as xtp,
            tc.tile_pool(name="etp", bufs=3) as etp,
            tc.tile_pool(name="dnp", bufs=2) as dnp,
            tc.tile_pool(name="outp", bufs=3) as outp,
            tc.tile_pool(name="ps_s", bufs=2, space="PSUM") as ps_s,
            tc.tile_pool(name="ps_y", bufs=2, space="PSUM") as ps_y,
            tc.tile_pool(name="ps_w", bufs=2, space="PSUM") as ps_w,
        ):
            wqk = big.tile([128, KO, 4, 128], bf16, name="wqk_sb")
            wv = big.tile([128, KO, DVE_], bf16, name="wv_sb")
            wo = big.tile([128, 2, D], bf16, name="wo_sb")
            bqk = big.tile([128, 4], f32, name="bqk_sb")
            bv = big.tile([128, LH, 65], f32, name="bv_sb")
            msk = big.tile([128, 4, TQC], bf16, name="msk_sb")
            # qk: subtile 0,1 = q head pairs (0,1),(2,3); 2,3 = k pairs.
            # within a subtile: even head on partitions 0:64, odd on 64:128
            qk = big.tile([128, 4, T], bf16, name="qk_sb")
            vsb = big.tile([128, 4 * NTQ, LH, 65], bf16, name="v_sb")
            yT = big.tile([128, 2, T], bf16, name="yT_sb")

            # ---- startup DMAs in first-need order, split across queues ----
            xcs = {}
            xcs[0] = xtp.tile([128, KO, TQC], bf16, name="xc_0", tag="xc")
            for ko in range(KO):
                nc.gpsimd.dma_start(wqk[:, ko], wqk_d[:, ko])
                nc.sync.dma_start(xcs[0][:, ko, :], xT_d[:, ko, 0:TQC])
            nc.gpsimd.dma_start(bqk[:], bqk_d[:])
            nc.gpsimd.dma_start(wv[:], wv_d[:])
            nc.gpsimd.dma_start(bv[:], bv_d[:])
            nc.gpsimd.dma_start(msk[:], msk_d[:])
            nc.gpsimd.dma_start(wo[:], wo_d[:])
            xcs[1] = xtp.tile([128, KO, TQC], bf16, name="xc_1", tag="xc")
            nc.sync.dma_start(xcs[1][:], xT_d[:, :, TQC : 2 * TQC])

            def emit_qk_group(c, s):
                xc = xcs[c]
                pst = ps_w.tile([128, TQC], f32, name=f"pqk_{c}_{s}", tag="work")
                for ko in range(KO):
                    nc.tensor.matmul(
                        pst[:],
                        wqk[:, ko, s, :],
                        xc[:, ko, :],
                        start=(ko == 0),
                        stop=(ko == KO - 1),
                    )
                nc.vector.tensor_scalar_add(
                    qk[:, s, ts(c, TQC)], pst[:], bqk[:, s : s + 1]
                )

            def emit_v_group(c, tbl):
                xc = xcs[c]
                tb = 4 * c + tbl
                pst = ps_w.tile([128, DVE_], f32, name=f"pv_{tb}", tag="work")
                for ko in range(KO):
                    nc.tensor.matmul(
                        pst[:],
                        xc[:, ko, ts(tbl, 128)],
                        wv[:, ko, :],
                        start=(ko == 0),
                        stop=(ko == KO - 1),
                    )
                nc.vector.tensor_add(
                    vsb[:, tb, :, :],
                    pst[:].rearrange("p (h e) -> p h e", h=LH),
                    bv[:],
                )

            def emit_out_group(c, m, n):
                pst = ps_w.tile([128, TQC], f32, name=f"po_{c}_{m}_{n}", tag="work")
                for kt in range(2):
                    nc.tensor.matmul(
                        pst[:],
                        yT[:, kt, ts(4 * c + m, 128)],
                        wo[:, kt, ts(n, TQC)],
                        start=(kt == 0),
                        stop=(kt == 1),
                    )
                ot = outp.tile([128, TQC], f32, name=f"ot_{c}_{m}_{n}", tag="ot")
                nc.vector.tensor_copy(ot[:], pst[:])
                nc.gpsimd.dma_start(out_d[ts(4 * c + m, 128), ts(n, TQC)], ot[:])

            bg = deque()

            def pump(k):
                for _ in range(min(k, len(bg))):
                    bg.popleft()()

            # ---- chunk 0 projections inline ----
            for s in range(4):
                emit_qk_group(0, s)
            for tbl in range(4):
                emit_v_group(0, tbl)

            for c in range(NTQ):
                cs = ts(c, TQC)
                nb = 4 * (c + 1)
                # prefetch x for chunk c+2
                if c + 2 < NTQ:
                    xcs[c + 2] = xtp.tile(
                        [128, KO, TQC], bf16, name=f"xc_{c+2}", tag="xc"
                    )
                    nc.sync.dma_start(
                        xcs[c + 2][:], xT_d[:, :, ts(c + 2, TQC)]
                    )
                # background PE work for this chunk's attention phase
                if c + 1 < NTQ:
                    for s in range(4):
                        bg.append(lambda c=c + 1, s=s: emit_qk_group(c, s))
                    for tbl in range(4):
                        bg.append(lambda c=c + 1, tbl=tbl: emit_v_group(c, tbl))
                if c >= 1:
                    for m in range(4):
                        for n in range(2):
                            bg.append(
                                lambda c=c - 1, m=m, n=n: emit_out_group(c, m, n)
                            )
                n_j = 2 * nb

                for p in range(2):  # head pair
                    psy = {}
                    for e in range(2):
                        psy[e] = ps_y.tile(
                            [128, TQC], f32, name=f"psy_{c}_{p}_{e}", tag="psy"
                        )
                    prev = None
                    for j in range(nb):
                        # scores for both heads of the pair (K=64 quadrants)
                        pss = ps_s.tile(
                            [128, 2, TQC], f32, name=f"pss_{c}_{p}_{j}", tag="pss"
                        )
                        for e in range(2):
                            pb = 64 * e
                            nc.tensor.matmul(
                                pss[:, e, :],
                                qk[pb : pb + 64, 2 + p, ts(j, 128)],
                                qk[pb : pb + 64, p, cs],
                                start=True,
                                stop=True,
                            )
                        et = etp.tile(
                            [128, 2, TQC], bf16, name=f"et_{c}_{p}_{j}", tag="et"
                        )
                        nc.scalar.activation(et[:], pss[:], AF.Exp, scale=1.0 / 32.0)
                        if j >= 4 * c:  # diagonal block: 0/1 causal mask
                            v_ = j - 4 * c
                            etm = etp.tile(
                                [128, 2, TQC], bf16, name=f"etm_{c}_{p}_{j}", tag="et"
                            )
                            for e in range(2):
                                nc.vector.tensor_mul(
                                    etm[:, e, :], et[:, e, :], msk[:, v_, :]
                                )
                            if debug_dumps and c == 0 and p == 0 and j == 0:
                                nc.sync.dma_start(dbg["et"][:], et[:])
                                nc.sync.dma_start(dbg["etm"][:], etm[:])
                            et = etm
                        # inject background projection / out-proj matmuls
                        rem_j = (1 - p) * nb + (nb - j)
                        pump(-(-len(bg) // max(rem_j, 1)))
                        if prev is not None:
                            pj, pet = prev
                            for e in range(2):
                                nc.tensor.matmul(
                                    psy[e][0:65, :],
                                    vsb[:, pj, 2 * p + e, :],
                                    pet[:, e, :],
                                    start=(pj == 0),
                                    stop=False,
                                )
                        prev = (j, et)
                    pj, pet = prev
                    for e in range(2):
                        nc.tensor.matmul(
                            psy[e][0:65, :],
                            vsb[:, pj, 2 * p + e, :],
                            pet[:, e, :],
                            start=(pj == 0),
                            stop=True,
                        )
                    # normalization: 1/denom (row 64) via ACT ln -> exp(-x)
                    # (reciprocal_approx_fast miscomputes on this hw; DVE
                    # InstReciprocal costs 3.3us per call)
                    for e in range(2):
                        dl = dnp.tile([128, TQC], f32, name=f"dl_{c}_{p}_{e}", tag="dl")
                        nc.scalar.activation(dl[64:65, :], psy[e][64:65, :], AF.Ln)
                        dh = dnp.tile([128, TQC], f32, name=f"dh_{c}_{p}_{e}", tag="dh")
                        nc.scalar.activation(
                            dh[64:65, :], dl[64:65, :], AF.Exp, scale=-1.0
                        )
                        d0 = dnp.tile([1, TQC], f32, name=f"d0_{c}_{p}_{e}", tag="d0")
                        nc.gpsimd.dma_start(d0[:], dh[64:65, :])
                        if debug_dumps:
                            idx = 4 * c + 2 * p + e
                            nc.sync.dma_start(dbg["d0"][idx : idx + 1, :], d0[:])
                        rb = dnp.tile([64, TQC], f32, name=f"rb_{c}_{p}_{e}", tag="rb")
                        nc.gpsimd.partition_broadcast(rb[:], d0[:])
                        if e == 0:
                            nc.vector.tensor_mul(
                                yT[0:64, p, cs], psy[e][0:64, :], rb[:]
                            )
                        else:
                            tmp = dnp.tile(
                                [64, TQC], bf16, name=f"tmp_{c}_{p}", tag="tmpy"
                            )
                            nc.vector.tensor_mul(tmp[:], psy[e][0:64, :], rb[:])
                            nc.gpsimd.dma_start(yT[64:128, p, cs], tmp[:])
                pump(len(bg))

            # final chunk's out projection
            for m in range(4):
                for n in range(2):
                    emit_out_group(NTQ - 1, m, n)

            if debug_dumps:
                nc.sync.dma_start(dbg["qk"][:], qk[:])
                nc.sync.dma_start(dbg["v"][:], vsb[:])
                nc.sync.dma_start(dbg["yT"][:], yT[:])

    nc.finalize()
    return nc


def _host_inputs(x, w_qkv, b_qkv, w_o, b_o):
    import ml_dtypes

    bf16 = ml_dtypes.bfloat16

    xT = []
    for b in range(B):
        t = np.ascontiguousarray(x[b].T)  # [D, T]
        xT.append(
            np.ascontiguousarray(
                t.reshape(KO, 128, T).swapaxes(0, 1).astype(bf16)
            )
        )

    p = np.arange(128)[:, None]
    f = np.arange(TQC)[None, :]
    masks = np.stack(
        [(f >= p + 128 * v).astype(np.float32) for v in range(4)], axis=1
    )  # [128, 4, 512]
    masks = np.ascontiguousarray(masks.astype(bf16))

    in_maps = []
    for core in range(8):
        b, g = divmod(core, 4)
        # qk groups: s=0,1 -> q head pairs; s=2,3 -> k head pairs
        wqk_g = np.zeros((D, 4, 128), dtype=np.float32)
        bqk_g = np.zeros((128, 4), dtype=np.float32)
        for s in range(4):
            base = 0 if s < 2 else D  # q vs k
            pair = s % 2
            cols = slice(
                base + g * 256 + pair * 128, base + g * 256 + pair * 128 + 128
            )
            wqk_g[:, s, :] = w_qkv[:, cols]
            bqk_g[:, s] = b_qkv[cols]
        wqk_g = np.ascontiguousarray(
            wqk_g.reshape(KO, 128, 4, 128).swapaxes(0, 1).astype(bf16)
        )

        w_v = np.zeros((D, DVE_), dtype=np.float32)
        b_v = np.zeros((LH, 65), dtype=np.float32)
        for h in range(LH):
            vcols = slice(2 * D + g * 256 + h * 64, 2 * D + g * 256 + (h + 1) * 64)
            w_v[:, h * 65 : h * 65 + 64] = w_qkv[:, vcols]
            b_v[h, 0:64] = b_qkv[vcols]
            b_v[h, 64] = 1.0  # ones column (weight col stays 0)
        w_v = np.ascontiguousarray(
            w_v.reshape(KO, 128, DVE_).swapaxes(0, 1).astype(bf16)
        )
        b_v_bc = np.ascontiguousarray(
            np.broadcast_to(b_v[None], (128, LH, 65)).copy()
        )

        w_o_g = w_o[g * 256 : (g + 1) * 256, :]  # [256, D]
        w_o_g = np.ascontiguousarray(
            w_o_g.reshape(2, 128, D).swapaxes(0, 1).astype(bf16)
        )

        in_maps.append(
            {
                "xT": xT[b],
                "w_qk": wqk_g,
                "b_qk": np.ascontiguousarray(bqk_g),
                "w_v": w_v,
                "b_v": b_v_bc,
                "w_o": w_o_g,
                "masks": masks,
            }
        )
    return in_maps


def kernel(x, w_qkv, b_qkv, w_o, b_o):
    global LAST_RESULT
    from concourse.bass_utils import run_bass_kernel_spmd

    x = np.asarray(x, dtype=np.float32)
    w_qkv = np.asarray(w_qkv, dtype=np.float32)
    b_qkv = np.asarray(b_qkv, dtype=np.float32)
    w_o = np.asarray(w_o, dtype=np.float32)
    b_o = np.asarray(b_o, dtype=np.float32)

    if "nc" not in _PROG:
        _PROG["nc"] = _build_program()
    nc = _PROG["nc"]

    in_maps = _host_inputs(x, w_qkv, b_qkv, w_o, b_o)

    trace = bool(os.environ.get("KERNEL_TRACE"))
    res = run_bass_kernel_spmd(nc, in_maps, core_ids=list(range(8)), trace=trace)
    LAST_RESULT = res

    out = np.empty((B, T, D), dtype=np.float32)
    for b in range(B):
        acc = res.results[b * 4]["out_part"].astype(np.float32).copy()
        for g in range(1, 4):
            acc += res.results[b * 4 + g]["out_part"]
        out[b] = acc + b_o[None, :]
    return out
